# revision 18
# baseline (speedup 1.0000x reference)
"""Bagging autoencoder ensemble kernel for 8 Trainium2 NeuronCores.

Strategy
--------
Batch-parallel: each core gets B/8 = 512 batch rows and computes all E=100
estimators on them. Host-side prep removes the gather entirely
(x[:, idx[e]] @ We0[e]  ==  x @ scatter_add(We0[e], idx[e])), folds the two
activation-free layers into their successors (W01 = W0s @ We1, Wzd1 = Wd0 @
Wd1 -- exact up to fp rounding since h0/d0 have no nonlinearity).

Numerics: the whole on-device chain runs on fp8-e4m3 operands (weights and
activations; fp32 PSUM accumulation), and the device stores PRE-sigmoid
activations as fp8; the host applies bias + sigmoid and transposes back.
Measured 1.0e-3 rel_l2 against the fp64 reference (gate 2e-2).

Pipeline design (v2) -- built for the HAM clock gate. The PE runs at 1.2GHz
cold and 2.4GHz warm; warmth requires the PE to stay ~90% busy, which means
the PSUM drains (ACT 1.2GHz / DVE 0.96GHz, the only engines with a PSUM
read port) must keep pace. Two phases:

  mid phase: all 39 mid-layer matmuls (7x L1 DoubleRow fp8 [K=256],
             7x z, 25x d1, each [128,512]) emitted software-pipelined in
             pairs per 2-bank PSUM tile; per-bank bias+relu drains
             alternate ACT/DVE and write fp8 activation tiles.
  out phase: 200 output matmuls ([64,128]x[64,512] block-diag fp8, two
             per 2-bank PSUM tile), one [128,1024] psum->fp8 drain per
             tile alternating ACT/DVE. One shared 4-buf psum pool gives
             the PE 4 tiles of runway so a 1.2us drain never stalls a
             ~0.5us tile fill. Drained fp8 goes to [128, 12*512] stage
             tiles, stored as 768KB DMAs to a flat bank-indexed output
             out[200,128,512] that the host unscrambles.

Matmul cost on TRN2 is (output free size) columns/cycle + ~60ns overhead;
PSUM writes are capped at one 2KB bank (N<=512) and operand slices must sit
at base partition 0/32/64, which dictates the pair/block-diag packing.
"""

import os
import sys

import numpy as np

for _p in ("/opt/trn_rl_repo", "/root/.axon_site/_ro/trn_rl_repo"):
    if os.path.isdir(_p) and _p not in sys.path:
        sys.path.append(_p)

import concourse.bass as bass
import concourse.mybir as mybir
import concourse.tile as tile
from concourse.bass_utils import run_bass_kernel_spmd

E, B, D, F, H, L = 100, 4096, 256, 32, 16, 8
N_CORES = 8
BC = B // N_CORES          # batch rows per core
G = 7                      # groups of 16 estimators (E padded 100 -> 112)
GE = 16
NQ = 25                    # real quads of 4 estimators (100 = 25*4 exactly)
NM = NQ * 8                # 200 output matmuls (q, pair, dq)
STB = 12                   # psum banks per staged store
F32 = mybir.dt.float32
BF16 = mybir.dt.bfloat16
F8 = mybir.dt.float8e4

OUT_PAT = "AD" * 12 + "A"  # output-tile drain engines (52/48 A:D balance)
MID_PAT = "DA"             # mid-layer drain engines


def _host_prep(x, idx, We0, be0, We1, be1, Wl, bl, Wd0, bd0, Wd1, bd1, Wo, bo):
    import ml_dtypes
    f32, f64 = np.float32, np.float64
    x = np.ascontiguousarray(np.asarray(x, f32))
    idx = np.asarray(idx).astype(np.int64)

    W0s = np.zeros((E, D, H), f64)
    We0_ = np.asarray(We0, f64)
    for e in range(E):
        np.add.at(W0s[e], idx[e], We0_[e])
    W01 = np.einsum('edh,ehl->edl', W0s, np.asarray(We1, f64))          # [E,256,8]
    b01 = np.einsum('eh,ehl->el', np.asarray(be0, f64),
                    np.asarray(We1, f64)) + np.asarray(be1, f64)        # [E,8]
    Wzd1 = np.einsum('elh,ehf->elf', np.asarray(Wd0, f64),
                     np.asarray(Wd1, f64))                              # [E,8,32]
    bzd1 = np.einsum('eh,ehf->ef', np.asarray(bd0, f64),
                     np.asarray(Wd1, f64)) + np.asarray(bd1, f64)       # [E,32]
    Wl_, bl_ = np.asarray(Wl, f32), np.asarray(bl, f32)
    Wo_ = np.asarray(Wo, f32)

    # group packing: partition p = 8*j + l for local est j (0..15), latent l
    w01 = np.zeros((128, G * 2 * 128), f32)
    b01g = np.zeros((128, G), f32)
    wbl = np.zeros((128, G * 128), f32)
    blg = np.zeros((128, G), f32)
    for g in range(G):
        for j in range(GE):
            e = g * GE + j
            if e >= E:
                continue
            for t in range(2):
                w01[:, (2 * g + t) * 128 + j * L:(2 * g + t) * 128 + (j + 1) * L] = \
                    W01[e, t * 128:(t + 1) * 128, :]
            b01g[j * L:(j + 1) * L, g] = b01[e]
            wbl[j * L:(j + 1) * L, g * 128 + j * L:g * 128 + (j + 1) * L] = Wl_[e]
            blg[j * L:(j + 1) * L, g] = bl_[e]

    # quad packing: d1 partition p = 32*jj + f for in-quad est jj, feature f.
    # The output layer runs per (quad, pair of est, d-quarter): block-diag
    # [64, 128] wo tiles so matmul operand slices stay at base partition 0/64.
    wzd1 = np.zeros((128, NQ * 128), f32)
    bd1q = np.zeros((128, NQ), f32)
    wo = np.zeros((128, NQ * 2 * 4 * 128), f32)
    for q in range(NQ):
        g, jloc0 = q // 4, (q % 4) * 4
        for jj in range(4):
            e = 4 * q + jj
            j = jloc0 + jj
            wzd1[j * L:(j + 1) * L, q * 128 + jj * F:q * 128 + (jj + 1) * F] = Wzd1[e]
            bd1q[jj * F:(jj + 1) * F, q] = bzd1[e]
            pair, a = jj // 2, jj % 2
            for dq in range(4):
                c = ((q * 2 + pair) * 4 + dq) * 128
                wo[64 * pair + 32 * a:64 * pair + 32 * (a + 1),
                   c + 64 * a:c + 64 * (a + 1)] = Wo_[e][:, dq * 64:(dq + 1) * 64]

    wo = wo.astype(ml_dtypes.float8_e4m3)

    # L1 runs as one fp8 DoubleRow matmul per group: x and W01 quantized to
    # e4m3 (measured rel_l2 impact: none), packed as [p, (ktile, col)].
    w01 = w01.astype(ml_dtypes.float8_e4m3)
    xts = [np.ascontiguousarray(
               x[c * BC:(c + 1) * BC, :].T.reshape(2, 128, BC)
               .transpose(1, 0, 2).reshape(128, 2 * BC))
           .astype(ml_dtypes.float8_e4m3)
           for c in range(N_CORES)]

    biases = np.concatenate([b01g, blg, bd1q], axis=1)   # [128, 2G+NQ]
    shared = dict(w01=w01, biases=biases, wo=wo,
                  wbl=wbl.astype(ml_dtypes.float8_e4m3),
                  wzd1=wzd1.astype(ml_dtypes.float8_e4m3))
    return shared, xts


def _legalize_waits(nc, max_waits=1):
    """This neuronxcc encodes a single sem-wait slot per instruction; hoist
    overflow waits onto same-engine NoOps placed immediately before."""
    ctr = 0
    for f in nc.m.functions:
        for bb in f.blocks:
            out = []
            for inst in bb.instructions:
                si = inst.sync_info
                if si is not None and si.on_wait and len(si.on_wait) > max_waits:
                    waits = list(si.on_wait)
                    extra, keep = waits[:-max_waits], waits[-max_waits:]
                    for j in range(0, len(extra), max_waits):
                        nop = mybir.InstNoOp(name=f"I-waitsplit-{ctr}")
                        ctr += 1
                        nop.engine = inst.engine
                        nop.sync_info = mybir.SyncInfo(
                            on_wait=extra[j:j + max_waits], on_update=[])
                        out.append(nop)
                    inst.sync_info = mybir.SyncInfo(
                        on_wait=keep, on_update=list(si.on_update or []))
                out.append(inst)
            bb.instructions[:] = out


def _build_nc(legalize=True):
    nc = bass.Bass("TRN2", target_bir_lowering=False, debug=False,
                   num_devices=N_CORES)
    xt_d = nc.declare_dram_parameter("xt", [128, 2 * BC], F8, isOutput=False)
    w01_d = nc.declare_dram_parameter("w01", [128, G * 2 * 128], F8, isOutput=False)
    bias_d = nc.declare_dram_parameter("biases", [128, 2 * G + NQ], F32,
                                       isOutput=False)
    wbl_d = nc.declare_dram_parameter("wbl", [128, G * 128], F8, isOutput=False)
    wzd1_d = nc.declare_dram_parameter("wzd1", [128, NQ * 128], F8, isOutput=False)
    wo_d = nc.declare_dram_parameter("wo", [128, NQ * 2 * 4 * 128], F8,
                                     isOutput=False)
    # flat bank-indexed output: m = (q*2+pair)*4 + dq, p = (a, d%64), batch;
    # partition-major [128, m*BC+b] so store DMAs run 6KB-contiguous lines
    out_d = nc.declare_dram_parameter("out", [128, NM * BC], F8, isOutput=True)

    ADD = mybir.AluOpType.add
    MAX = mybir.AluOpType.max
    RELU = mybir.ActivationFunctionType.Relu
    COPY = mybir.ActivationFunctionType.Copy

    DR = mybir.MatmulPerfMode.DoubleRow
    # first-chunk sizes (groups 0-1 / quads 0-7) so compute starts early
    WBLA, WZA, WOA = 2 * 128, 8 * 128, 8 * 2 * 4 * 128

    with tile.TileContext(nc) as tc:
        with (
            tc.tile_pool(name="const", bufs=1) as cp,
            tc.tile_pool(name="acts", bufs=1) as acts,
            tc.tile_pool(name="stage", bufs=3) as stp,
            tc.tile_pool(name="ps", bufs=4, space="PSUM") as psp,
        ):
            # ---- input loads on the SP ring, earliest-needed first; the
            # tiny bias pack leads so drains are never bias-gated
            bias_t = cp.tile([128, 2 * G + NQ], F32, tag="biases")
            nc.sync.dma_start(out=bias_t[:], in_=bias_d[:, :])
            xt8 = cp.tile([128, 2 * BC], F8, tag="xt8")
            nc.sync.dma_start(out=xt8[:], in_=xt_d[:, :])
            w018_t = cp.tile([128, G * 2 * 128], F8, tag="w018")
            nc.sync.dma_start(out=w018_t[:], in_=w01_d[:, :])
            wbla_t = cp.tile([128, WBLA], F8, tag="wbla")
            nc.sync.dma_start(out=wbla_t[:], in_=wbl_d[:, :WBLA])
            # mid-phase weights stay on the Sync queue (small, needed
            # first); the big output-layer weights ride the otherwise idle
            # GPSIMD queue so neither queue blocks the other
            wblb_t = cp.tile([128, G * 128 - WBLA], F8, tag="wblb")
            nc.sync.dma_start(out=wblb_t[:], in_=wbl_d[:, WBLA:])
            wza_t = cp.tile([128, WZA], F8, tag="wza")
            nc.sync.dma_start(out=wza_t[:], in_=wzd1_d[:, :WZA])
            wzb_t = cp.tile([128, NQ * 128 - WZA], F8, tag="wzb")
            nc.sync.dma_start(out=wzb_t[:], in_=wzd1_d[:, WZA:])
            woa_t = cp.tile([128, WOA], F8, tag="woa")
            nc.gpsimd.dma_start(out=woa_t[:], in_=wo_d[:, :WOA])
            wob_t = cp.tile([128, NQ * 2 * 4 * 128 - WOA], F8, tag="wob")
            nc.gpsimd.dma_start(out=wob_t[:], in_=wo_d[:, WOA:])
            # prime the ACT activation table while loads are in flight so
            # the first real RELU doesn't pay the ~1.3us ACT_TABLE_LOAD
            warm_t = cp.tile([128, 1], F32, tag="warm")
            nc.scalar.activation(warm_t[:], bias_t[:, :1], RELU)

            def wbl_sl(g):
                c = g * 128
                return wbla_t[:, c:c + 128] if c < WBLA else \
                    wblb_t[:, c - WBLA:c - WBLA + 128]

            def wz_sl(q):
                c = q * 128
                return wza_t[:, c:c + 128] if c < WZA else \
                    wzb_t[:, c - WZA:c - WZA + 128]

            def wo_sl(m):
                c = m * 128
                wt = woa_t if c < WOA else wob_t
                c = c if c < WOA else c - WOA
                pair = (m // 4) % 2
                return wt[64 * pair:64 * (pair + 1), c:c + 128]

            h1s, zs, d1s = {}, {}, {}
            mid_n = [0]

            def mid_drain_engine():
                e = MID_PAT[mid_n[0] % len(MID_PAT)]
                mid_n[0] += 1
                return e

            def emit_mid(job, pso):
                kind, i = job
                if kind == "l1":
                    # one fp8 DoubleRow matmul: both 128-row K-tiles of the
                    # folded 256-dim contraction stream together
                    lhsT = w018_t[:, i * 256:(i + 1) * 256].rearrange(
                        "p (two m) -> p two m", two=2, m=128)
                    rhs = xt8[:].rearrange("p (two b) -> p two b", two=2, b=BC)
                    nc.tensor.matmul(pso, lhsT, rhs, start=True, stop=True,
                                     perf_mode=DR)
                    dst = acts.tile([128, BC], F8, tag=f"h1_{i}")
                    bias = bias_t[:, i:i + 1]
                    h1s[i] = dst
                elif kind == "z":
                    nc.tensor.matmul(pso, wbl_sl(i), h1s[i][:],
                                     start=True, stop=True)
                    dst = acts.tile([128, BC], F8, tag=f"z_{i}")
                    bias = bias_t[:, G + i:G + i + 1]
                    zs[i] = dst
                else:  # d1 quad i
                    nc.tensor.matmul(pso, wz_sl(i), zs[i // 4][:],
                                     start=True, stop=True)
                    dst = acts.tile([128, BC], F8, tag=f"d1_{i}")
                    bias = bias_t[:, 2 * G + i:2 * G + i + 1]
                    d1s[i] = dst
                if mid_drain_engine() == "A":
                    nc.scalar.activation(dst[:], pso, RELU, bias=bias)
                else:
                    nc.vector.tensor_scalar(dst[:], pso, bias, 0.0, ADD, MAX)

            # mid-layer job sequence, software-pipelined so each matmul's
            # data dep (h1 for z, z for d1) was drained >=2 matmuls earlier.
            seq = [("l1", 0), ("l1", 1), ("l1", 2), ("z", 0),
                   ("z", 1), ("d1", 0), ("z", 2), ("d1", 1),
                   ("l1", 3), ("d1", 2), ("d1", 3), ("z", 3),
                   ("d1", 4), ("l1", 4), ("d1", 5), ("d1", 6),
                   ("z", 4), ("d1", 7), ("l1", 5), ("d1", 8),
                   ("d1", 9), ("z", 5), ("d1", 10), ("l1", 6),
                   ("d1", 11), ("d1", 12), ("z", 6), ("d1", 13),
                   ("d1", 14)]
            seq += [("d1", q) for q in range(15, NQ)]
            assert len(seq) == G + G + NQ
            assert sorted(i for k, i in seq if k == "d1") == list(range(NQ))

            # ---- merged emission: mid pair-tiles interleaved with output
            # tiles (2 matmuls + one [128,1024] ACT/DVE drain each) as soon
            # as their d1 quad is drained; 12-bank staged 6KB-line stores.
            stage_st = [None, 0, 0]   # tile, base m, filled tiles
            out_n = [0]

            def emit_out_tile(t):
                m = 2 * t
                if stage_st[0] is None:
                    stage_st[0] = stp.tile([128, STB * BC], F8, tag="stage",
                                           name="stage_t")
                    stage_st[1] = m
                stage_t, sbase = stage_st[0], stage_st[1]
                pso = psp.tile([128, 2 * BC], F32, tag="ps")
                for k in range(2):
                    mm = m + k
                    nc.tensor.matmul(pso[:, k * BC:(k + 1) * BC], wo_sl(mm),
                                     d1s[mm // 8][64 * ((mm // 4) % 2):
                                                  64 * ((mm // 4) % 2 + 1), :],
                                     start=True, stop=True)
                sl = stage_t[:, (m - sbase) * BC:(m - sbase + 2) * BC]
                if OUT_PAT[out_n[0] % len(OUT_PAT)] == "A":
                    nc.scalar.activation(sl, pso[:], COPY)
                else:
                    nc.vector.tensor_scalar(sl, pso[:], 0.0, None, ADD)
                out_n[0] += 1
                if m + 2 - sbase == STB or m + 2 == NM:
                    nb = m + 2 - sbase
                    nc.sync.dma_start(
                        out=out_d.ap()[:, sbase * BC:(sbase + nb) * BC],
                        in_=stage_t[:, :nb * BC])
                    stage_st[0] = None

            # two-phase: FIFO drain queues mean interleaving output tiles
            # into the mid phase delays mid drains ~2.3us behind output
            # drains and head-of-line-stalls the in-order PE queue. Each
            # mid matmul gets its own 1-bank tile: tile-granular dep
            # tracking would otherwise serialize a pair's second matmul
            # behind the first one's drain.
            for job in seq:
                pst = psp.tile([128, 2 * BC], F32, tag="ps", name="pst")
                emit_mid(job, pst[:, :BC])
            for t in range(NM // 2):
                emit_out_tile(t)

    if legalize:
        _legalize_waits(nc)
    return nc


_NC_CACHE = []


def kernel(x, idx, We0, be0, We1, be1, Wl, bl, Wd0, bd0, Wd1, bd1, Wo, bo,
           _trace=False, _trace_cores=None):
    shared, xts = _host_prep(x, idx, We0, be0, We1, be1, Wl, bl,
                             Wd0, bd0, Wd1, bd1, Wo, bo)
    if not _NC_CACHE:
        _NC_CACHE.append(_build_nc())
    nc = _NC_CACHE[0]
    in_maps = [dict(shared, xt=xts[c]) for c in range(N_CORES)]
    res = run_bass_kernel_spmd(nc, in_maps, list(range(N_CORES)),
                               trace=_trace, trace_cores=_trace_cores)
    # host epilogue: fp8 pre-sigmoid [(a,dd), m=(q,pair,dq), b] -> [E,B,D]
    raw = np.stack([np.asarray(res.results[c]["out"]) for c in range(N_CORES)])
    pre = raw.astype(np.float32).reshape(N_CORES, 2, 64, NQ, 2, 4, BC)
    pre = pre.transpose(0, 3, 4, 1, 5, 2, 6).reshape(N_CORES, E, D, BC)
    pre = np.moveaxis(pre, 0, 2).reshape(E, D, B)          # [E, D, B]
    pre += np.asarray(bo, np.float32)[:, :, None]
    out = np.ascontiguousarray(
        (1.0 / (1.0 + np.exp(-pre))).transpose(0, 2, 1))   # [E, B, D]
    if _trace:
        return out, res
    return out


# revision 28
# speedup vs baseline: 1.1211x; 1.1211x over previous
"""Bagging autoencoder ensemble kernel for 8 Trainium2 NeuronCores.

Strategy
--------
Batch-parallel: each core gets B/8 = 512 batch rows and computes all E=100
estimators on them. Host-side prep removes the gather entirely
(x[:, idx[e]] @ We0[e]  ==  x @ scatter_add(We0[e], idx[e])), folds the two
activation-free layers into their successors (W01 = W0s @ We1, Wzd1 = Wd0 @
Wd1 -- exact up to fp rounding since h0/d0 have no nonlinearity).

Numerics: the whole on-device chain runs on fp8-e4m3 operands (weights and
activations; fp32 PSUM accumulation), and the device stores PRE-sigmoid
activations as fp8; the host applies bias + sigmoid and transposes back.
Measured 1.0e-3 rel_l2 against the fp64 reference (gate 2e-2).

Pipeline design (v2) -- built for the HAM clock gate. The PE runs at 1.2GHz
cold and 2.4GHz warm; warmth requires the PE to stay ~90% busy, which means
the PSUM drains (ACT 1.2GHz / DVE 0.96GHz, the only engines with a PSUM
read port) must keep pace. Two phases:

  mid phase: all 39 mid-layer matmuls (7x L1 DoubleRow fp8 [K=256],
             7x z, 25x d1, each [128,512]) emitted software-pipelined in
             pairs per 2-bank PSUM tile; per-bank bias+relu drains
             alternate ACT/DVE and write fp8 activation tiles.
  out phase: 200 output matmuls ([64,128]x[64,512] block-diag fp8, two
             per 2-bank PSUM tile), one [128,1024] psum->fp8 drain per
             tile alternating ACT/DVE. One shared 4-buf psum pool gives
             the PE 4 tiles of runway so a 1.2us drain never stalls a
             ~0.5us tile fill. Drained fp8 goes to [128, 12*512] stage
             tiles, stored as 768KB DMAs to a flat bank-indexed output
             out[200,128,512] that the host unscrambles.

Matmul cost on TRN2 is (output free size) columns/cycle + ~60ns overhead;
PSUM writes are capped at one 2KB bank (N<=512) and operand slices must sit
at base partition 0/32/64, which dictates the pair/block-diag packing.
"""

import os
import sys

import numpy as np

for _p in ("/opt/trn_rl_repo", "/root/.axon_site/_ro/trn_rl_repo"):
    if os.path.isdir(_p) and _p not in sys.path:
        sys.path.append(_p)

import concourse.bass as bass
import concourse.mybir as mybir
import concourse.tile as tile
from concourse.bass_utils import run_bass_kernel_spmd

E, B, D, F, H, L = 100, 4096, 256, 32, 16, 8
N_CORES = 8
BC = B // N_CORES          # batch rows per core
G = 7                      # groups of 16 estimators (E padded 100 -> 112)
GE = 16
NQ = 25                    # real quads of 4 estimators (100 = 25*4 exactly)
NM = NQ * 8                # 200 output matmuls (q, pair, dq)
STB = 12                   # psum banks per staged store
F32 = mybir.dt.float32
BF16 = mybir.dt.bfloat16
F8 = mybir.dt.float8e4

OUT_PAT = "AD" * 12 + "A"  # output-tile drain engines (52/48 A:D balance)
MID_PAT = "DA"             # mid-layer drain engines


def _host_prep(x, idx, We0, be0, We1, be1, Wl, bl, Wd0, bd0, Wd1, bd1, Wo, bo):
    import ml_dtypes
    f32, f64 = np.float32, np.float64
    x = np.ascontiguousarray(np.asarray(x, f32))
    idx = np.asarray(idx).astype(np.int64)

    W0s = np.zeros((E, D, H), f64)
    We0_ = np.asarray(We0, f64)
    for e in range(E):
        np.add.at(W0s[e], idx[e], We0_[e])
    W01 = np.einsum('edh,ehl->edl', W0s, np.asarray(We1, f64))          # [E,256,8]
    b01 = np.einsum('eh,ehl->el', np.asarray(be0, f64),
                    np.asarray(We1, f64)) + np.asarray(be1, f64)        # [E,8]
    Wzd1 = np.einsum('elh,ehf->elf', np.asarray(Wd0, f64),
                     np.asarray(Wd1, f64))                              # [E,8,32]
    bzd1 = np.einsum('eh,ehf->ef', np.asarray(bd0, f64),
                     np.asarray(Wd1, f64)) + np.asarray(bd1, f64)       # [E,32]
    Wl_, bl_ = np.asarray(Wl, f32), np.asarray(bl, f32)
    Wo_ = np.asarray(Wo, f32)

    # group packing: partition p = 8*j + l for local est j (0..15), latent l
    w01 = np.zeros((128, G * 2 * 128), f32)
    b01g = np.zeros((128, G), f32)
    wbl = np.zeros((128, G * 128), f32)
    blg = np.zeros((128, G), f32)
    for g in range(G):
        for j in range(GE):
            e = g * GE + j
            if e >= E:
                continue
            for t in range(2):
                w01[:, (2 * g + t) * 128 + j * L:(2 * g + t) * 128 + (j + 1) * L] = \
                    W01[e, t * 128:(t + 1) * 128, :]
            b01g[j * L:(j + 1) * L, g] = b01[e]
            wbl[j * L:(j + 1) * L, g * 128 + j * L:g * 128 + (j + 1) * L] = Wl_[e]
            blg[j * L:(j + 1) * L, g] = bl_[e]

    # quad packing: d1 partition p = 32*jj + f for in-quad est jj, feature f.
    # The output layer runs per (est, d-half): one DENSE [32,128] wo tile at
    # base partition 32*jj (matmul tile_position allows 0/32/64/96 for
    # 32-row operands), so wo carries no block-diag zero padding at all.
    wzd1 = np.zeros((128, NQ * 128), f32)
    bd1q = np.zeros((128, NQ), f32)
    wo = np.zeros((128, NQ * 256), f32)
    for q in range(NQ):
        g, jloc0 = q // 4, (q % 4) * 4
        for jj in range(4):
            e = 4 * q + jj
            j = jloc0 + jj
            wzd1[j * L:(j + 1) * L, q * 128 + jj * F:q * 128 + (jj + 1) * F] = Wzd1[e]
            bd1q[jj * F:(jj + 1) * F, q] = bzd1[e]
            wo[32 * jj:32 * (jj + 1), q * 256:(q + 1) * 256] = Wo_[e]

    wo = wo.astype(ml_dtypes.float8_e4m3)

    # L1 runs as one fp8 DoubleRow matmul per group: x and W01 quantized to
    # e4m3 (measured rel_l2 impact: none), packed as [p, (ktile, col)].
    w01 = w01.astype(ml_dtypes.float8_e4m3)
    xts = [np.ascontiguousarray(
               x[c * BC:(c + 1) * BC, :].T.reshape(2, 128, BC)
               .transpose(1, 0, 2).reshape(128, 2 * BC))
           .astype(ml_dtypes.float8_e4m3)
           for c in range(N_CORES)]

    biases = np.concatenate([b01g, blg, bd1q], axis=1)   # [128, 2G+NQ]
    shared = dict(w01=w01, biases=biases, wo=wo,
                  wbl=wbl.astype(ml_dtypes.float8_e4m3),
                  wzd1=wzd1.astype(ml_dtypes.float8_e4m3))
    return shared, xts


def _legalize_waits(nc, max_waits=1):
    """This neuronxcc encodes a single sem-wait slot per instruction; hoist
    overflow waits onto same-engine NoOps placed immediately before."""
    ctr = 0
    for f in nc.m.functions:
        for bb in f.blocks:
            out = []
            for inst in bb.instructions:
                si = inst.sync_info
                if si is not None and si.on_wait and len(si.on_wait) > max_waits:
                    waits = list(si.on_wait)
                    extra, keep = waits[:-max_waits], waits[-max_waits:]
                    for j in range(0, len(extra), max_waits):
                        nop = mybir.InstNoOp(name=f"I-waitsplit-{ctr}")
                        ctr += 1
                        nop.engine = inst.engine
                        nop.sync_info = mybir.SyncInfo(
                            on_wait=extra[j:j + max_waits], on_update=[])
                        out.append(nop)
                    inst.sync_info = mybir.SyncInfo(
                        on_wait=keep, on_update=list(si.on_update or []))
                out.append(inst)
            bb.instructions[:] = out


def _build_nc(legalize=True):
    nc = bass.Bass("TRN2", target_bir_lowering=False, debug=False,
                   num_devices=N_CORES)
    xt_d = nc.declare_dram_parameter("xt", [128, 2 * BC], F8, isOutput=False)
    w01_d = nc.declare_dram_parameter("w01", [128, G * 2 * 128], F8, isOutput=False)
    bias_d = nc.declare_dram_parameter("biases", [128, 2 * G + NQ], F32,
                                       isOutput=False)
    wbl_d = nc.declare_dram_parameter("wbl", [128, G * 128], F8, isOutput=False)
    wzd1_d = nc.declare_dram_parameter("wzd1", [128, NQ * 128], F8, isOutput=False)
    wo_d = nc.declare_dram_parameter("wo", [128, NQ * 256], F8,
                                     isOutput=False)
    # flat bank-indexed output: m = 2*est + d-half, p = d%128, batch;
    # partition-major [128, m*BC+b] so store DMAs run 6KB-contiguous lines
    out_d = nc.declare_dram_parameter("out", [128, NM * BC], F8, isOutput=True)

    ADD = mybir.AluOpType.add
    MAX = mybir.AluOpType.max
    RELU = mybir.ActivationFunctionType.Relu
    COPY = mybir.ActivationFunctionType.Copy

    DR = mybir.MatmulPerfMode.DoubleRow
    # first-chunk sizes (groups 0-1 / quads 0-7) so compute starts early
    WBLA, WZA, WOA = 2 * 128, 8 * 128, 8 * 256

    with tile.TileContext(nc) as tc:
        with (
            tc.tile_pool(name="const", bufs=1) as cp,
            tc.tile_pool(name="acts", bufs=1) as acts,
            tc.tile_pool(name="stage", bufs=5) as stp,
            tc.tile_pool(name="ps", bufs=4, space="PSUM") as psp,
        ):
            # ---- input loads on the SP ring, earliest-needed first; the
            # tiny bias pack leads so drains are never bias-gated
            bias_t = cp.tile([128, 2 * G + NQ], F32, tag="biases")
            nc.sync.dma_start(out=bias_t[:], in_=bias_d[:, :])
            xt8 = cp.tile([128, 2 * BC], F8, tag="xt8")
            nc.sync.dma_start(out=xt8[:], in_=xt_d[:, :])
            w018_t = cp.tile([128, G * 2 * 128], F8, tag="w018")
            nc.sync.dma_start(out=w018_t[:], in_=w01_d[:, :])
            wbla_t = cp.tile([128, WBLA], F8, tag="wbla")
            nc.sync.dma_start(out=wbla_t[:], in_=wbl_d[:, :WBLA])
            # mid-phase weights stay on the Sync queue (small, needed
            # first); the big output-layer weights ride the otherwise idle
            # GPSIMD queue so neither queue blocks the other
            wblb_t = cp.tile([128, G * 128 - WBLA], F8, tag="wblb")
            nc.sync.dma_start(out=wblb_t[:], in_=wbl_d[:, WBLA:])
            wza_t = cp.tile([128, WZA], F8, tag="wza")
            nc.sync.dma_start(out=wza_t[:], in_=wzd1_d[:, :WZA])
            wzb_t = cp.tile([128, NQ * 128 - WZA], F8, tag="wzb")
            nc.sync.dma_start(out=wzb_t[:], in_=wzd1_d[:, WZA:])
            woa_t = cp.tile([128, WOA], F8, tag="woa")
            nc.gpsimd.dma_start(out=woa_t[:], in_=wo_d[:, :WOA])
            wob_t = cp.tile([128, NQ * 256 - WOA], F8, tag="wob")
            nc.gpsimd.dma_start(out=wob_t[:], in_=wo_d[:, WOA:])
            # prime the ACT activation table while loads are in flight so
            # the first real RELU doesn't pay the ~1.3us ACT_TABLE_LOAD
            warm_t = cp.tile([128, 1], F32, tag="warm")
            nc.scalar.activation(warm_t[:], bias_t[:, :1], RELU)

            def wbl_sl(g):
                c = g * 128
                return wbla_t[:, c:c + 128] if c < WBLA else \
                    wblb_t[:, c - WBLA:c - WBLA + 128]

            def wz_sl(q):
                c = q * 128
                return wza_t[:, c:c + 128] if c < WZA else \
                    wzb_t[:, c - WZA:c - WZA + 128]

            def wo_sl(m):
                # m = 2*e + h (est, d-half): dense [32,128] at partition 32*jj
                e, h = m // 2, m % 2
                q, jj = e // 4, e % 4
                c = q * 256 + h * 128
                wt = woa_t if c < WOA else wob_t
                c = c if c < WOA else c - WOA
                return wt[32 * jj:32 * (jj + 1), c:c + 128]

            h1s, zs, d1s = {}, {}, {}
            mid_n = [0]

            def mid_drain_engine():
                e = MID_PAT[mid_n[0] % len(MID_PAT)]
                mid_n[0] += 1
                return e

            def emit_mid(job, pso):
                kind, i = job
                if kind == "l1":
                    # one fp8 DoubleRow matmul: both 128-row K-tiles of the
                    # folded 256-dim contraction stream together
                    lhsT = w018_t[:, i * 256:(i + 1) * 256].rearrange(
                        "p (two m) -> p two m", two=2, m=128)
                    rhs = xt8[:].rearrange("p (two b) -> p two b", two=2, b=BC)
                    nc.tensor.matmul(pso, lhsT, rhs, start=True, stop=True,
                                     perf_mode=DR)
                    dst = acts.tile([128, BC], F8, tag=f"h1_{i}")
                    bias = bias_t[:, i:i + 1]
                    h1s[i] = dst
                elif kind == "z":
                    nc.tensor.matmul(pso, wbl_sl(i), h1s[i][:],
                                     start=True, stop=True)
                    dst = acts.tile([128, BC], F8, tag=f"z_{i}")
                    bias = bias_t[:, G + i:G + i + 1]
                    zs[i] = dst
                else:  # d1 quad i
                    nc.tensor.matmul(pso, wz_sl(i), zs[i // 4][:],
                                     start=True, stop=True)
                    dst = acts.tile([128, BC], F8, tag=f"d1_{i}")
                    bias = bias_t[:, 2 * G + i:2 * G + i + 1]
                    d1s[i] = dst
                if mid_drain_engine() == "A":
                    nc.scalar.activation(dst[:], pso, RELU, bias=bias)
                else:
                    nc.vector.tensor_scalar(dst[:], pso, bias, 0.0, ADD, MAX)

            # mid-layer job sequence, software-pipelined so each matmul's
            # data dep (h1 for z, z for d1) was drained >=2 matmuls earlier.
            seq = [("l1", 0), ("l1", 1), ("l1", 2), ("z", 0),
                   ("z", 1), ("d1", 0), ("z", 2), ("d1", 1),
                   ("l1", 3), ("d1", 2), ("d1", 3), ("z", 3),
                   ("d1", 4), ("l1", 4), ("d1", 5), ("d1", 6),
                   ("z", 4), ("d1", 7), ("l1", 5), ("d1", 8),
                   ("d1", 9), ("z", 5), ("d1", 10), ("l1", 6),
                   ("d1", 11), ("d1", 12), ("z", 6), ("d1", 13),
                   ("d1", 14)]
            seq += [("d1", q) for q in range(15, NQ)]
            assert len(seq) == G + G + NQ
            assert sorted(i for k, i in seq if k == "d1") == list(range(NQ))

            # ---- merged emission: mid pair-tiles interleaved with output
            # tiles (2 matmuls + one [128,1024] ACT/DVE drain each) as soon
            # as their d1 quad is drained; 12-bank staged 6KB-line stores.
            stage_st = [None, 0, 0]   # tile, base m, filled tiles
            out_n = [0]

            def emit_out_tile(t):
                m = 2 * t
                if stage_st[0] is None:
                    stage_st[0] = stp.tile([128, STB * BC], F8, tag="stage",
                                           name="stage_t")
                    stage_st[1] = m
                stage_t, sbase = stage_st[0], stage_st[1]
                pso = psp.tile([128, 2 * BC], F32, tag="ps")
                for k in range(2):
                    mm = m + k
                    e = mm // 2
                    q, jj = e // 4, e % 4
                    nc.tensor.matmul(pso[:, k * BC:(k + 1) * BC], wo_sl(mm),
                                     d1s[q][32 * jj:32 * (jj + 1), :],
                                     start=True, stop=True,
                                     tile_position=(32 * jj, 0))
                sl = stage_t[:, (m - sbase) * BC:(m - sbase + 2) * BC]
                if OUT_PAT[out_n[0] % len(OUT_PAT)] == "A":
                    nc.scalar.activation(sl, pso[:], COPY)
                else:
                    nc.vector.tensor_scalar(sl, pso[:], 0.0, None, ADD)
                out_n[0] += 1
                if m + 2 - sbase == STB or m + 2 == NM:
                    nb = m + 2 - sbase
                    nc.sync.dma_start(
                        out=out_d.ap()[:, sbase * BC:(sbase + nb) * BC],
                        in_=stage_t[:, :nb * BC])
                    stage_st[0] = None

            # two-phase: FIFO drain queues mean interleaving output tiles
            # into the mid phase delays mid drains ~2.3us behind output
            # drains and head-of-line-stalls the in-order PE queue. Each
            # mid matmul gets its own 1-bank tile: tile-granular dep
            # tracking would otherwise serialize a pair's second matmul
            # behind the first one's drain.
            for job in seq:
                pst = psp.tile([128, 2 * BC], F32, tag="ps", name="pst")
                emit_mid(job, pst[:, :BC])
            for t in range(NM // 2):
                emit_out_tile(t)

    if legalize:
        _legalize_waits(nc)
    return nc


_NC_CACHE = []


def kernel(x, idx, We0, be0, We1, be1, Wl, bl, Wd0, bd0, Wd1, bd1, Wo, bo,
           _trace=False, _trace_cores=None):
    shared, xts = _host_prep(x, idx, We0, be0, We1, be1, Wl, bl,
                             Wd0, bd0, Wd1, bd1, Wo, bo)
    if not _NC_CACHE:
        _NC_CACHE.append(_build_nc())
    nc = _NC_CACHE[0]
    in_maps = [dict(shared, xt=xts[c]) for c in range(N_CORES)]
    res = run_bass_kernel_spmd(nc, in_maps, list(range(N_CORES)),
                               trace=_trace, trace_cores=_trace_cores)
    # host epilogue: fp8 pre-sigmoid [dd, m=(est,half), b] -> [E,B,D]
    raw = np.stack([np.asarray(res.results[c]["out"]) for c in range(N_CORES)])
    pre = raw.astype(np.float32).reshape(N_CORES, 128, E, 2, BC)
    pre = pre.transpose(0, 2, 3, 1, 4).reshape(N_CORES, E, D, BC)
    pre = np.moveaxis(pre, 0, 2).reshape(E, D, B)          # [E, D, B]
    pre += np.asarray(bo, np.float32)[:, :, None]
    out = np.ascontiguousarray(
        (1.0 / (1.0 + np.exp(-pre))).transpose(0, 2, 1))   # [E, B, D]
    if _trace:
        return out, res
    return out
